# revision 3
# baseline (speedup 1.0000x reference)
"""Trainium2 Bass kernel for the C. elegans Hodgkin-Huxley network simulation.

Strategy
--------
Pure data parallel over the worm/batch axis: 512 worms -> 64 per NeuronCore
across 8 cores.  State is transposed ([neuron-slot, batch]) packed into flat
[128, 192] SBUF tiles (3 chunks of 128 slots; neurons permuted so chemical
presynaptic neurons sit in chunk 0 and gap-junction neurons in chunk 1).

Per step the update is reorganised for the cost model:
 - vtrap(x,y) = y*(w + w/tanh(w)) with w = x/(2y): tanh comes from the same
   ACT table as exp, w/tanh(w) >= 1 always, and a tensor max with a constant
   1.0 tile exactly repairs the removable 0/0 singularity (w and the tanh
   argument are computed with identical scale/bias so the fp16 roundings
   cancel in the ratio near w = 0).
 - gate state m,h,n stays fp32 (fp16 state drifts past the tolerance); the
   A/B rate tensors and the ionic m^3 h / n^4 chain are fp16, picking up the
   DVE 2x/4x wide modes ((mult,add) tensor_scalar combos and 2-byte
   tensor_tensor).
 - the V update runs through one PSUM accumulation:
       ps = I16@It + (c1*I)@V + Wg@V_gap + Wc@tanh-act + I@KCn
   and V' = V*KV2n + ps in two DVE ops.  KCn/K3n stay fp32 (their magnitude
   makes fp16 rounding a ~0.05 mV/step V error).
 - engine balance: affine preps on Pool, squares on ACT, the rest on DVE.
"""

import os
import numpy as np

# Model constants (must match the reference)
N = 302
DT = 0.05
G_CHEM = 0.1
G_GAP = 0.05
G_NA, E_NA = 120.0, 50.0
G_K, E_K = 36.0, -77.0
G_L, E_L = 0.3, -54.387

B_FULL = 512
T_FULL = 256
N_CORES = 8
BL = B_FULL // N_CORES      # 64 worms per core
NS = 384                    # padded neuron-slot count (3 chunks of 128)
NCH = 3                     # chunks
FW = NCH * BL               # 192 = flat free width of one step's state

_CACHE = {}


def _hh_init_gates():
    V0 = -65.0
    def vtrap(x, y):
        return x / -np.expm1(-x / y)
    a_m = 0.1 * vtrap(V0 + 40.0, 10.0)
    b_m = 4.0 * np.exp(-(V0 + 65.0) / 18.0)
    a_h = 0.07 * np.exp(-(V0 + 65.0) / 20.0)
    b_h = 1.0 / (1.0 + np.exp(-(V0 + 35.0) / 10.0))
    a_n = 0.01 * vtrap(V0 + 55.0, 10.0)
    b_n = 0.125 * np.exp(-(V0 + 65.0) / 80.0)
    m0 = a_m / (a_m + b_m)
    h0 = a_h / (a_h + b_h)
    n0 = a_n / (a_n + b_n)
    return float(m0), float(h0), float(n0)


def _build_program(T, TC):
    """Build + compile the SPMD Bass program (one core's view)."""
    import concourse.bacc as bacc
    import concourse.mybir as mybir
    import concourse.tile as tile
    from concourse._compat import get_trn_type

    f32 = mybir.dt.float32
    f16 = mybir.dt.float16
    op = mybir.AluOpType
    AF = mybir.ActivationFunctionType

    NB = T // TC
    m0, h0, n0 = _hh_init_gates()

    nc = bacc.Bacc(get_trn_type() or "TRN2", target_bir_lowering=False,
                   debug=False, num_devices=N_CORES)

    it_d = nc.dram_tensor("it_in", [NB, NCH, 128, TC, BL], f16,
                          kind="ExternalInput")
    wc_d = nc.dram_tensor("w_chem", [128, NS], f16, kind="ExternalInput")
    wg_d = nc.dram_tensor("w_gap", [128, NS], f32, kind="ExternalInput")
    wi16_d = nc.dram_tensor("w_id16", [128, 128], f16, kind="ExternalInput")
    wic1_d = nc.dram_tensor("w_c1", [128, 128], f32, kind="ExternalInput")
    wik_d = nc.dram_tensor("w_id32", [128, 128], f32, kind="ExternalInput")
    v_d = nc.dram_tensor("v_out", [NB, NCH, 128, TC, BL], f32,
                         kind="ExternalOutput")

    # activation-op constants
    AH_B = float(np.log(0.07 * DT) - 65.0 / 20.0)
    BM_B = float(np.log(4.0 * DT) - 65.0 / 18.0)
    BN_B = float(np.log(0.125 * DT) - 65.0 / 80.0)

    with tile.TileContext(nc) as tc_:
        with (
            tc_.tile_pool(name="persist", bufs=1) as pp,
            tc_.tile_pool(name="gst", bufs=2) as gp,
            tc_.tile_pool(name="io", bufs=2) as iop,
            tc_.tile_pool(name="scr", bufs=3) as sp,
            tc_.tile_pool(name="psum", bufs=8, space="PSUM") as psp,
        ):
            # --- persistent tiles --------------------------------------------
            Wc = pp.tile([128, NS], f16, tag="wc")
            Wg = pp.tile([128, NS], f32, tag="wg")
            Wi16 = pp.tile([128, 128], f16, tag="wi16")
            WiC1 = pp.tile([128, 128], f32, tag="wic1")
            WiK = pp.tile([128, 128], f32, tag="wik")
            Vinit = pp.tile([128, FW], f32, tag="vinit")
            ONES2 = pp.tile([128, 2 * FW], f16, tag="ones2")
            SCL2 = pp.tile([128, 2 * FW], f16, tag="scl2")

            nc.sync.dma_start(Wc[:], wc_d.ap())
            nc.sync.dma_start(Wg[:], wg_d.ap())
            nc.sync.dma_start(Wi16[:], wi16_d.ap())
            nc.sync.dma_start(WiC1[:], wic1_d.ap())
            nc.sync.dma_start(WiK[:], wik_d.ap())
            nc.gpsimd.memset(Vinit[:], -65.0)
            nc.gpsimd.memset(ONES2[:], 1.0)
            nc.gpsimd.memset(SCL2[:, 0:FW], 0.05)       # m vtrap: DT*0.1*10
            nc.gpsimd.memset(SCL2[:, FW:2 * FW], 0.005)  # n vtrap: DT*0.01*10

            # gate state [h|m|n], fp32
            G0 = gp.tile([128, 3 * FW], f32, tag="g")
            nc.gpsimd.memset(G0[:, 0:FW], h0)
            nc.gpsimd.memset(G0[:, FW:2 * FW], m0)
            nc.gpsimd.memset(G0[:, 2 * FW:3 * FW], n0)

            # per-partition bias constants for the ACT ops
            bias_vals = [2.0, 2.75, 1.75, AH_B, BM_B, BN_B]
            bias_ap = {}
            bias_tile = pp.tile([128, len(bias_vals)], f32, tag="biases")
            for i, bv in enumerate(bias_vals):
                nc.gpsimd.memset(bias_tile[:, i:i + 1], bv)
                bias_ap[bv] = bias_tile[:, i:i + 1]

            Vap = Vinit[:]
            Gap = G0[:]
            for ib in range(NB):
                itb = iop.tile([128, TC * FW], f16, tag="itb")
                outb = iop.tile([128, TC * FW], f32, tag="outb")
                itb_r = itb[:].rearrange("p (t c b) -> p t c b",
                                         t=TC, c=NCH, b=BL)
                for c in range(NCH):
                    nc.sync.dma_start(itb_r[:, :, c, :], it_d.ap()[ib, c])

                for tt in range(TC):
                    Gh = Gap[:, 0:FW]
                    Gm = Gap[:, FW:2 * FW]
                    Gmn = Gap[:, FW:3 * FW]

                    # ============ ACT rates from V ===========================
                    A3 = sp.tile([128, 3 * FW], f16, tag="A3")   # [Ah|Am|An]
                    B3 = sp.tile([128, 3 * FW], f16, tag="B3")   # [Bh|Bm|Bn]
                    Tw2 = sp.tile([128, 2 * FW], f16, tag="Tw2")  # [Twm|Twn]
                    Th = sp.tile([128, FW], f16, tag="Th")
                    Tc = sp.tile([128, BL], f16, tag="Tc")
                    nc.scalar.activation(Tc[:], Vap[:, 0:BL], AF.Tanh,
                                         bias=bias_ap[2.0], scale=0.1)
                    nc.scalar.activation(A3[:, 0:FW], Vap, AF.Exp,
                                         bias=bias_ap[AH_B], scale=-0.05)
                    nc.scalar.activation(B3[:, FW:2 * FW], Vap, AF.Exp,
                                         bias=bias_ap[BM_B], scale=float(-1 / 18))
                    nc.scalar.activation(B3[:, 2 * FW:3 * FW], Vap, AF.Exp,
                                         bias=bias_ap[BN_B], scale=float(-1 / 80))
                    nc.scalar.activation(Tw2[:, 0:FW], Vap, AF.Tanh,
                                         bias=bias_ap[2.0], scale=0.05)
                    nc.scalar.activation(Tw2[:, FW:2 * FW], Vap, AF.Tanh,
                                         bias=bias_ap[2.75], scale=0.05)
                    nc.scalar.activation(Th[:], Vap, AF.Tanh,
                                         bias=bias_ap[1.75], scale=0.05)

                    # ============ vtrap via tanh-coth ========================
                    w2 = sp.tile([128, 2 * FW], f16, tag="w2")
                    nc.gpsimd.tensor_scalar(w2[:, 0:FW], Vap, 0.05, 2.0,
                                            op.mult, op.add)
                    nc.gpsimd.tensor_scalar(w2[:, FW:2 * FW], Vap, 0.05, 2.75,
                                            op.mult, op.add)
                    rat = sp.tile([128, 2 * FW], f16, tag="rat")
                    ratr = sp.tile([128, 2 * FW], f16, tag="ratr")
                    sum2 = sp.tile([128, 2 * FW], f16, tag="sum2")
                    rcpT = sp.tile([128, 2 * FW], f32, tag="rcpT")
                    nc.vector.reciprocal(rcpT[:], Tw2[:])
                    nc.vector.tensor_tensor(rat[:], w2[:], rcpT[:], op.mult)
                    nc.vector.tensor_tensor(ratr[:], rat[:], ONES2[:], op.max)
                    nc.vector.scalar_tensor_tensor(sum2[:], w2[:], 1.0,
                                                   ratr[:], op.mult, op.add)
                    nc.vector.tensor_tensor(A3[:, FW:3 * FW], sum2[:], SCL2[:],
                                            op.mult)
                    nc.gpsimd.tensor_scalar(B3[:, 0:FW], Th[:], 0.025, 0.025,
                                            op.mult, op.add)

                    # ============ gate update (fp32 state) ===================
                    Aq = sp.tile([128, 3 * FW], f16, tag="Aq")
                    R = sp.tile([128, 3 * FW], f16, tag="R")
                    W2t = sp.tile([128, 3 * FW], f32, tag="W2t")
                    Gn = gp.tile([128, 3 * FW], f32, tag="g")
                    nc.vector.tensor_scalar(Aq[:], A3[:], -1.0, 1.0,
                                            op.mult, op.add)
                    nc.vector.scalar_tensor_tensor(R[:], B3[:], -1.0, Aq[:],
                                                   op.mult, op.add)
                    nc.vector.tensor_tensor(W2t[:], Gap[:], R[:], op.mult)
                    nc.vector.scalar_tensor_tensor(Gn[:], A3[:], 1.0, W2t[:],
                                                   op.mult, op.add)

                    # ============ ionic chain (fp16, fp32 K-consts) ==========
                    sq2 = sp.tile([128, 2 * FW], f16, tag="sq2")  # [m^2|n^2]
                    nc.scalar.square(sq2[:], Gmn)
                    m6n = sp.tile([128, FW], f16, tag="m6n")
                    nc.gpsimd.tensor_scalar(m6n[:], Gm, -6.0, 0.0,
                                            op.mult, op.add)
                    mh6 = sp.tile([128, FW], f16, tag="mh6")
                    nc.vector.tensor_tensor(mh6[:], m6n[:], Gh, op.mult)
                    I6n = sp.tile([128, FW], f16, tag="I6n")
                    nc.vector.tensor_tensor(I6n[:], sq2[:, 0:FW], mh6[:],
                                            op.mult)
                    n4 = sp.tile([128, FW], f16, tag="n4")
                    nc.vector.tensor_tensor(n4[:], sq2[:, FW:2 * FW],
                                            sq2[:, FW:2 * FW], op.mult)
                    KV2n = sp.tile([128, FW], f16, tag="KV2n")
                    nc.vector.scalar_tensor_tensor(KV2n[:], n4[:], -1.8,
                                                   I6n[:], op.mult, op.add)
                    K3n = sp.tile([128, FW], f32, tag="K3n")
                    nc.gpsimd.tensor_scalar(K3n[:], n4[:], -138.6, 0.0,
                                            op.mult, op.add)
                    KCn = sp.tile([128, FW], f32, tag="KCn")
                    nc.vector.scalar_tensor_tensor(KCn[:], I6n[:], -50.0,
                                                   K3n[:], op.mult, op.add)
                    W3n = sp.tile([128, FW], f32, tag="W3n")
                    nc.vector.tensor_tensor(W3n[:], Vap, KV2n[:], op.mult)

                    # ============ PE: psum accumulation ======================
                    ps = psp.tile([128, FW], f32, tag="ps")
                    it0 = tt * FW
                    nc.tensor.matmul(ps[:], Wi16[:], itb[:, it0:it0 + FW],
                                     start=True, stop=False)
                    nc.tensor.matmul(ps[:], WiC1[:], Vap, start=False,
                                     stop=False)
                    for mi in range(NCH):
                        cr = slice(mi * BL, (mi + 1) * BL)
                        nc.tensor.matmul(ps[:, cr], Wg[:, mi * 128:(mi + 1) * 128],
                                         Vap[:, BL:2 * BL], start=False,
                                         stop=False, skip_group_check=True)
                    for mi in range(NCH):
                        cr = slice(mi * BL, (mi + 1) * BL)
                        nc.tensor.matmul(ps[:, cr], Wc[:, mi * 128:(mi + 1) * 128],
                                         Tc[:], start=False, stop=False,
                                         skip_group_check=True)
                    nc.tensor.matmul(ps[:], WiK[:], KCn[:], start=False,
                                     stop=True)

                    # ============ V update ===================================
                    vout = outb[:, tt * FW:(tt + 1) * FW]
                    nc.vector.scalar_tensor_tensor(vout, W3n[:], 1.0, ps[:],
                                                   op.mult, op.add)
                    Gap = Gn[:]
                    Vap = vout

                outb_r = outb[:].rearrange("p (t c b) -> p t c b",
                                           t=TC, c=NCH, b=BL)
                for c in range(NCH):
                    nc.sync.dma_start(v_d.ap()[ib, c], outb_r[:, :, c, :])

    nc.compile()
    return nc


def _get_program(T=T_FULL, TC=16):
    key = (T, TC)
    if key not in _CACHE:
        _CACHE[key] = _build_program(T, TC)
    return _CACHE[key]


def _prep_weights(chem, gap):
    """Neuron permutation + folded weight matrices (float64 host prep)."""
    chem = np.asarray(chem, np.float64)
    gap = np.asarray(gap, np.float64)
    gap_eff = gap - np.diag(gap.sum(axis=0))

    p_chem = np.nonzero(np.any(chem != 0.0, axis=1))[0]
    p_gap = np.nonzero(np.any(gap_eff != 0.0, axis=1))[0]
    assert len(p_chem) <= 128, f"chem pre-set {len(p_chem)} > 128"
    assert len(p_gap) <= 128, f"gap set {len(p_gap)} > 128"

    set_c, set_g = set(p_chem.tolist()), set(p_gap.tolist())
    rest = [n for n in range(N) if n not in set_c and n not in set_g]

    chunk0 = list(p_chem)
    fill = list(rest)
    while len(chunk0) < 128:
        chunk0.append(fill.pop())
    used = set(chunk0)

    chunk1 = list(p_gap)
    rem = [n for n in range(N) if n not in used and n not in set_g]
    while len(chunk1) < 128 and rem:
        chunk1.append(rem.pop())
    used |= set(chunk1)

    chunk2 = [n for n in range(N) if n not in used]
    assert len(chunk1) <= 128 and len(chunk2) <= 128
    slots = np.full(NS, -1, np.int64)
    slots[0:128] = chunk0
    slots[128:128 + len(chunk1)] = chunk1
    slots[256:256 + len(chunk2)] = chunk2

    live = slots >= 0
    slot_of = np.full(N, -1, np.int64)
    for s in range(NS - 1, -1, -1):
        if slots[s] >= 0:
            slot_of[slots[s]] = s
    assert (slot_of >= 0).all()

    # col[n, s] = 1 iff slots[s] == n
    col = np.zeros((N, NS), np.float64)
    col[slots[live], np.nonzero(live)[0]] = 1.0

    Wc = (0.5 * DT * G_CHEM) * (chem[np.array(chunk0)] @ col)          # [128, NS]
    Wg = (DT * G_GAP) * (gap_eff[np.array(slots[128:256].clip(min=0))] @ col)
    dead1 = ~live[128:256]
    Wg[dead1] = 0.0

    # per-neuron additive constant: leak offset + chem sigmoid 0.5-offset
    Cn = DT * G_L * E_L + (0.5 * DT * G_CHEM) * chem.sum(axis=0)       # [N]

    return (Wc.astype(np.float16), Wg.astype(np.float32),
            Cn, slots, slot_of, live)


def kernel(I_ext, chem_weights, gap_weights):
    from concourse.bass_utils import run_bass_kernel_spmd

    I_ext = np.asarray(I_ext, np.float32)
    B, T, Nn = I_ext.shape
    assert (B, T, Nn) == (B_FULL, T_FULL, N)

    Wc, Wg, Cn, slots, slot_of, live = _prep_weights(
        np.asarray(chem_weights, np.float32),
        np.asarray(gap_weights, np.float32))

    TC = 16 if T_FULL % 16 == 0 else 1
    NB = T_FULL // TC
    nc = _get_program(T_FULL, TC)

    c1 = 1.0 - DT * G_L
    live_idx = np.nonzero(live)[0]
    nrn = slots[live_idx]

    in_maps = []
    for c in range(N_CORES):
        I_loc = I_ext[c * BL:(c + 1) * BL]                 # [BL, T, N]
        arr = np.zeros((NS, T, BL), np.float16)
        vals = (DT * np.transpose(I_loc, (2, 1, 0))[nrn].astype(np.float64)
                + Cn[nrn, None, None])
        arr[live_idx] = vals.astype(np.float16)
        it_blk = np.transpose(
            arr.reshape(NCH, 128, NB, TC, BL), (2, 0, 1, 3, 4))
        in_maps.append({
            "it_in": np.ascontiguousarray(it_blk),
            "w_chem": Wc, "w_gap": Wg,
            "w_id16": np.eye(128, dtype=np.float16),
            "w_c1": (c1 * np.eye(128)).astype(np.float32),
            "w_id32": np.eye(128, dtype=np.float32),
        })

    trace = bool(os.environ.get("KERNEL_TRACE"))
    res = run_bass_kernel_spmd(nc, in_maps, list(range(N_CORES)), trace=trace)
    globals()["LAST_RESULTS"] = res

    out = np.empty((B_FULL, T_FULL, N), np.float32)
    for c in range(N_CORES):
        vb = res.results[c]["v_out"].reshape(NB, NCH, 128, TC, BL)
        vd = np.transpose(vb, (1, 2, 0, 3, 4)).reshape(NS, T_FULL, BL)
        out[c * BL:(c + 1) * BL] = np.transpose(vd[slot_of], (2, 1, 0))
    return out
